# revision 9
# baseline (speedup 1.0000x reference)
"""Tensor-parallel GQA attention prefill for 8 TRN2 NeuronCores.

Sharding: each core owns 4 query heads + 1 kv head (column-shard of
wq/wk/wv by head) and a 512-row slice of wo's input dim (row-shard).
Each core computes a partial output projection over its local heads;
the host sums the 8 partials (equivalent to the all-reduce in the
sharding hint) and transposes back to [b, s, d].

Device math (per core), all layouts feature-on-partitions:
  qT/kT/vT = W^T-tile.T @ xT-tile accumulated over 32 d-tiles (fp32r)
  RoPE applied in "rotate_half" form: weight rows are pre-permuted on
  the host (even features first, then odd) so the pair (2i, 2i+1)
  becomes (i, i+64) and the cross-partition shuffle is two 64-partition
  copies instead of a stride-2 partition gather.
  scores[tq,tk] per (b,h) via matmul over head_dim, +mask, softmax on
  the free axis, PE-transpose of probs, PV matmul, then the wo matmul
  over local features only (partial sums, bf16).
"""

import math
from contextlib import ExitStack

import ml_dtypes
import numpy as np

import concourse.bass as bass
import concourse.tile as tile
from concourse import bacc, mybir
from concourse.bass_utils import run_bass_kernel_spmd

DIM = 4096
N_HEADS = 32
HEAD_DIM = 128
N_KV_HEADS = 8
BSZ = 4
SEQLEN = 128
T = BSZ * SEQLEN  # 512 tokens
NCORES = 8
HQ = N_HEADS // NCORES  # 4 query heads per core
EQ = HQ * HEAD_DIM  # 512 local q features
ND = DIM // 128  # 32 contraction tiles
SCALE = 1.0 / math.sqrt(HEAD_DIM)

F32 = mybir.dt.float32
F32R = mybir.dt.float32r
BF16 = mybir.dt.bfloat16
AX = mybir.AxisListType
ACTF = mybir.ActivationFunctionType
PSUM = bass.MemorySpace.PSUM

_STATE: dict = {}
LAST_RESULT = None


def _install_ntff_hook():
    """Register the axon NTFF profile hook if the image lacks antenv.axon_hooks.

    Lets run_bass_kernel_spmd(trace=True) return exec_time_ns + perfetto
    under axon. Best-effort: any failure leaves tracing disabled but the
    kernel still runs.
    """
    import os
    import sys
    import types

    try:
        import antenv.axon_hooks  # noqa: F401

        return
    except ImportError:
        pass
    try:
        import antenv
        from trn_agent_boot.trn_boot import _ntff_profile_via_ctypes

        mod = types.ModuleType("antenv.axon_hooks")
        holder = {"hook": None}
        mod.set_axon_ntff_profile_hook = lambda h: holder.__setitem__("hook", h)
        mod.get_axon_ntff_profile_hook = lambda: holder["hook"]
        sys.modules["antenv.axon_hooks"] = mod
        antenv.axon_hooks = mod
        so = "/opt/axon/libaxon_pjrt.so"
        if os.path.exists(so):
            hook = _ntff_profile_via_ctypes(so)
            if hook is not None:
                mod.set_axon_ntff_profile_hook(hook)
    except Exception:
        pass


_install_ntff_hook()


def _build_nc():
    nc = bacc.Bacc(
        "TRN2",
        target_bir_lowering=False,
        debug=False,
        enable_asserts=False,
        num_devices=NCORES,
    )
    xT = nc.dram_tensor("xT", [DIM, T], F32R, kind="ExternalInput").ap()
    # head-major: rows h*DIM + d, cols = the head's 128 features
    wqT = nc.dram_tensor("wqT", [HQ * DIM, HEAD_DIM], F32R, kind="ExternalInput").ap()
    wkT = nc.dram_tensor("wkT", [DIM, HEAD_DIM], F32R, kind="ExternalInput").ap()
    wvT = nc.dram_tensor("wvT", [DIM, HEAD_DIM], F32R, kind="ExternalInput").ap()
    woT = nc.dram_tensor("woT", [EQ, DIM], BF16, kind="ExternalInput").ap()
    mask4 = nc.dram_tensor("mask4", [128, T], F32, kind="ExternalInput").ap()
    cq = nc.dram_tensor("cq", [128, T], F32, kind="ExternalInput").ap()
    sq = nc.dram_tensor("sq", [128, T], F32, kind="ExternalInput").ap()
    ck = nc.dram_tensor("ck", [128, T], F32, kind="ExternalInput").ap()
    sk = nc.dram_tensor("sk", [128, T], F32, kind="ExternalInput").ap()
    ident = nc.dram_tensor("ident", [128, 128], BF16, kind="ExternalInput").ap()
    yT = nc.dram_tensor("yT", [DIM, T], BF16, kind="ExternalOutput").ap()

    with tile.TileContext(nc) as tc, ExitStack() as ctx:
        const = ctx.enter_context(tc.tile_pool(name="const", bufs=1))
        wqp = ctx.enter_context(tc.tile_pool(name="wqp", bufs=2))
        qtp = ctx.enter_context(tc.tile_pool(name="qtp", bufs=2))
        rt = ctx.enter_context(tc.tile_pool(name="rt", bufs=2))
        sm = ctx.enter_context(tc.tile_pool(name="sm", bufs=3))
        yp = ctx.enter_context(tc.tile_pool(name="yp", bufs=4))
        ps = ctx.enter_context(tc.tile_pool(name="ps", bufs=8, space=PSUM))

        # ---- PE warm-up: dense dummy matmuls while the first DMAs land ----
        warm_w = const.tile([128, 128], BF16, tag="warm_w")
        nc.gpsimd.memset(warm_w[:], 0.0)
        warm_x = const.tile([128, T], BF16, tag="warm_x")
        nc.gpsimd.memset(warm_x[:], 0.0)
        ps_warm = ps.tile([128, T], F32, tag="ps")
        for _ in range(20):
            nc.tensor.matmul(ps_warm[:], warm_w[:], warm_x[:], start=True, stop=True)

        # ---- input DMAs ----
        # x resident (re-read by the K/V sweep and all 4 Q sweeps); ramped
        # group sizes so the first sweep matmuls start early. sync ring: x,
        # wo, half the outputs; scalar ring: wk/wv, wq heads, tables.
        x_sb = const.tile([128, ND, T], F32R, tag="x")
        XGROUPS = [2, 2, 4, 8, 8, 8]
        j0 = 0
        for gd in XGROUPS:
            rows = slice(j0 * 128, (j0 + gd) * 128)
            nc.sync.dma_start(
                x_sb[:, j0 : j0 + gd, :],
                xT[rows, :].rearrange("(j p) t -> p j t", p=128),
            )
            j0 += gd
        wk_sb = const.tile([128, ND, HEAD_DIM], F32R, tag="wk")
        wv_sb = const.tile([128, ND, HEAD_DIM], F32R, tag="wv")
        for g in range(4):
            rows = slice(g * 8 * 128, (g + 1) * 8 * 128)
            nc.scalar.dma_start(
                wk_sb[:, g * 8 : (g + 1) * 8, :],
                wkT[rows, :].rearrange("(j p) e -> p j e", p=128),
            )
            nc.scalar.dma_start(
                wv_sb[:, g * 8 : (g + 1) * 8, :],
                wvT[rows, :].rearrange("(j p) e -> p j e", p=128),
            )
        ident_sb = const.tile([128, 128], BF16, tag="ident")
        nc.scalar.dma_start(ident_sb[:], ident)
        ck_sb = const.tile([128, T], F32, tag="ck")
        nc.scalar.dma_start(ck_sb[:], ck)
        sk_sb = const.tile([128, T], F32, tag="sk")
        nc.scalar.dma_start(sk_sb[:], sk)
        wq_tiles = []
        for h in range(HQ):
            wqt = wqp.tile([128, ND, HEAD_DIM], F32R, tag="wq", name=f"wq{h}")
            nc.scalar.dma_start(
                wqt[:],
                wqT[h * DIM : (h + 1) * DIM, :].rearrange("(j p) e -> p j e", p=128),
            )
            wq_tiles.append(wqt)
            if h == 0:
                cq_sb = const.tile([128, T], F32, tag="cq")
                nc.scalar.dma_start(cq_sb[:], cq)
                sq_sb = const.tile([128, T], F32, tag="sq")
                nc.scalar.dma_start(sq_sb[:], sq)
                mask_sb = const.tile([128, T], F32, tag="mask")
                nc.scalar.dma_start(mask_sb[:], mask4)
        wo_sb = const.tile([128, HQ * DIM], BF16, tag="wo")
        nc.sync.dma_start(
            wo_sb[:].rearrange("p (j c) -> p j c", j=HQ),
            woT.rearrange("(j p) c -> p j c", p=128),
        )

        kT_sb = const.tile([128, T], F32, tag="kT")
        vT_sb = const.tile([128, T], BF16, tag="vT")
        v_sb = const.tile([128, BSZ * HEAD_DIM], BF16, tag="v")
        oT_sb = const.tile([128, HQ * T], BF16, tag="oT")

        def rope(dst_ap, pssrc, ctab, stab):
            swp = rt.tile([128, T], F32, tag="swp")
            nc.scalar.copy(swp[0:64, :], pssrc[64:128, :])
            nc.scalar.copy(swp[64:128, :], pssrc[0:64, :])
            prod = rt.tile([128, T], F32, tag="prod")
            nc.vector.tensor_mul(prod[:], pssrc[:], ctab)
            nc.vector.tensor_mul(swp[:], swp[:], stab)
            nc.vector.tensor_add(dst_ap, prod[:], swp[:])

        # ---- K/V sweep (rides the incoming x stream) ----
        ps_k = ps.tile([128, T], F32, tag="ps")
        ps_v = ps.tile([128, T], F32, tag="ps")
        for j in range(ND):
            st, sp = (j == 0), (j == ND - 1)
            nc.tensor.matmul(ps_k[:], wk_sb[:, j, :], x_sb[:, j, :], start=st, stop=sp)
            nc.tensor.matmul(ps_v[:], wv_sb[:, j, :], x_sb[:, j, :], start=st, stop=sp)
        rope(kT_sb[:], ps_k[:], ck_sb[:], sk_sb[:])
        nc.scalar.copy(vT_sb[:], ps_v[:])
        for b in range(BSZ):
            bs = slice(b * 128, (b + 1) * 128)
            ps_t = ps.tile([128, T], BF16, tag="ps")
            nc.tensor.transpose(ps_t[:, 0:128], vT_sb[:, bs], ident_sb[:])
            nc.vector.tensor_copy(v_sb[:, bs], ps_t[:, 0:128])

        # ---- per-head Q sweep; head h's attention PE ops are emitted after
        # head h+1's sweep so the PE never stalls on softmax latency ----
        def q_sweep(h):
            ps_qh = ps.tile([128, T], F32, tag="ps", name=f"ps_q{h}")
            wqt = wq_tiles[h]
            for j in range(ND):
                st, sp = (j == 0), (j == ND - 1)
                nc.tensor.matmul(
                    ps_qh[:], wqt[:, j, :], x_sb[:, j, :], start=st, stop=sp
                )
            qt = qtp.tile([128, T], F32, tag="qT", name=f"qT{h}")
            rope(qt[:], ps_qh[:], cq_sb[:], sq_sb[:])
            return qt

        def attention(h, qt):
            ps_s = ps.tile([128, T], F32, tag="ps", name=f"ps_s{h}")
            for b in range(BSZ):
                bs = slice(b * 128, (b + 1) * 128)
                nc.tensor.matmul(
                    ps_s[:, bs], qt[:, bs], kT_sb[:, bs], start=True, stop=True
                )
            s_sb = sm.tile([128, T], F32, tag="s")
            nc.vector.tensor_add(s_sb[:], ps_s[:], mask_sb[:])
            nmx = sm.tile([128, BSZ], F32, tag="nmx")
            den = sm.tile([128, BSZ], F32, tag="den")
            rden = sm.tile([128, BSZ], F32, tag="rden")
            p_sb = sm.tile([128, T], BF16, tag="p")
            for b in range(BSZ):
                bs = slice(b * 128, (b + 1) * 128)
                nc.vector.reduce_max(
                    nmx[:, b : b + 1], s_sb[:, bs], axis=AX.X, negate=True
                )
                nc.scalar.activation(
                    p_sb[:, bs],
                    s_sb[:, bs],
                    ACTF.Exp,
                    bias=nmx[:, b : b + 1],
                    accum_out=den[:, b : b + 1],
                )
            nc.vector.reciprocal(rden[:], den[:])
            for b in range(BSZ):
                bs = slice(b * 128, (b + 1) * 128)
                nc.vector.tensor_scalar_mul(p_sb[:, bs], p_sb[:, bs], rden[:, b : b + 1])
            for b in range(BSZ):
                bs = slice(b * 128, (b + 1) * 128)
                ps_pt = ps.tile([128, T], BF16, tag="ps", name=f"ps_pt{h}_{b}")
                nc.tensor.transpose(ps_pt[:, 0:128], p_sb[:, bs], ident_sb[:])
                pt_sb = sm.tile([128, 128], BF16, tag="pt")
                nc.scalar.copy(pt_sb[:], ps_pt[:, 0:128])
                ps_o = ps.tile([128, T], F32, tag="ps", name=f"ps_o{h}_{b}")
                nc.tensor.matmul(
                    ps_o[:, 0:128], v_sb[:, bs], pt_sb[:], start=True, stop=True
                )
                nc.vector.tensor_copy(
                    oT_sb[:, h * T + b * 128 : h * T + (b + 1) * 128], ps_o[:, 0:128]
                )

        prev = None
        for h in range(HQ):
            qt = q_sweep(h)
            if prev is not None:
                attention(h - 1, prev)
            prev = qt
        attention(HQ - 1, prev)

        # ---- output projection over local features (partial sums) ----
        for dt in range(ND):
            ps_y = ps.tile([128, T], F32, tag="ps", name=f"ps_y{dt}")
            for j in range(HQ):
                nc.tensor.matmul(
                    ps_y[:],
                    wo_sb[:, j * DIM + dt * 128 : j * DIM + (dt + 1) * 128],
                    oT_sb[:, j * T : (j + 1) * T],
                    start=(j == 0),
                    stop=(j == HQ - 1),
                )
            y_sb = yp.tile([128, T], BF16, tag="y", name=f"y{dt}")
            if dt % 2 == 0:
                nc.vector.tensor_copy(y_sb[:], ps_y[:])
                nc.sync.dma_start(yT[dt * 128 : (dt + 1) * 128, :], y_sb[:])
            else:
                nc.scalar.copy(y_sb[:], ps_y[:])
                nc.scalar.dma_start(yT[dt * 128 : (dt + 1) * 128, :], y_sb[:])

    nc.compile()
    return nc


def get_nc():
    if "nc" not in _STATE:
        _STATE["nc"] = _build_nc()
    return _STATE["nc"]


def _prep_in_maps(x, wq, wk, wv, wo, freqs_cos, freqs_sin, mask):
    f32 = np.float32
    x = np.asarray(x, f32)
    wq = np.asarray(wq, f32)
    wk = np.asarray(wk, f32)
    wv = np.asarray(wv, f32)
    wo = np.asarray(wo, f32)
    fc = np.asarray(freqs_cos, f32)
    fs = np.asarray(freqs_sin, f32)
    mask = np.asarray(mask, f32)

    # even features first, then odd: (2i, 2i+1) pairs -> (i, i+64)
    perm = np.concatenate([np.arange(0, HEAD_DIM, 2), np.arange(1, HEAD_DIM, 2)])
    wqp = wq.reshape(N_HEADS, HEAD_DIM, DIM)[:, perm, :].reshape(DIM, DIM)
    wkp = wk.reshape(N_KV_HEADS, HEAD_DIM, DIM)[:, perm, :].reshape(
        N_KV_HEADS * HEAD_DIM, DIM
    )

    xT = np.ascontiguousarray(x.reshape(T, DIM).T)
    C0 = np.vstack([fc.T, fc.T])  # [128, 128]: row p -> cos[t, p % 64]
    S0 = np.vstack([-fs.T, fs.T])
    cq = np.ascontiguousarray(np.tile(C0 * SCALE, (1, BSZ)))
    sq = np.ascontiguousarray(np.tile(S0 * SCALE, (1, BSZ)))
    ck = np.ascontiguousarray(np.tile(C0, (1, BSZ)))
    sk = np.ascontiguousarray(np.tile(S0, (1, BSZ)))
    mask4 = np.ascontiguousarray(np.tile(mask[0, 0], (1, BSZ)))
    ident = np.eye(128, dtype=ml_dtypes.bfloat16)

    in_maps = []
    for c in range(NCORES):
        qrows = slice(c * EQ, (c + 1) * EQ)
        krows = slice(c * HEAD_DIM, (c + 1) * HEAD_DIM)
        in_maps.append(
            {
                "xT": xT,
                "wqT": np.ascontiguousarray(
                    wqp[qrows, :].reshape(HQ, HEAD_DIM, DIM).transpose(0, 2, 1).reshape(HQ * DIM, HEAD_DIM)
                ),
                "wkT": np.ascontiguousarray(wkp[krows, :].T),
                "wvT": np.ascontiguousarray(wv[krows, :].T),
                "woT": np.ascontiguousarray(wo[:, qrows].T).astype(ml_dtypes.bfloat16),
                "mask4": mask4,
                "cq": cq,
                "sq": sq,
                "ck": ck,
                "sk": sk,
                "ident": ident,
            }
        )
    return in_maps


def kernel(
    x,
    wq,
    wk,
    wv,
    wo,
    cache_k,
    cache_v,
    freqs_cos,
    freqs_sin,
    mask,
    start_pos,
    *,
    trace=False,
    trace_kwargs=None,
):
    global LAST_RESULT
    sp = int(np.asarray(start_pos))
    assert sp == 0, f"kernel specialized for start_pos=0, got {sp}"

    in_maps = _prep_in_maps(x, wq, wk, wv, wo, freqs_cos, freqs_sin, mask)
    nc = get_nc()
    res = run_bass_kernel_spmd(
        nc,
        in_maps,
        core_ids=list(range(NCORES)),
        trace=trace,
        **(trace_kwargs or {}),
    )
    LAST_RESULT = res
    acc = np.zeros((DIM, T), np.float32)
    for c in range(NCORES):
        acc += res.results[c]["yT"].astype(np.float32)
    return np.ascontiguousarray(acc.T).reshape(BSZ, SEQLEN, DIM)


# revision 12
# speedup vs baseline: 1.4485x; 1.4485x over previous
"""Tensor-parallel GQA attention prefill for 8 TRN2 NeuronCores.

Sharding: each core owns 4 query heads + 1 kv head (column-shard of
wq/wk/wv by head) and a 512-row slice of wo's input dim (row-shard).
Each core computes a partial output projection over its local heads;
the host sums the 8 partials (equivalent to the all-reduce in the
sharding hint) and transposes back to [b, s, d].

Device math (per core), all layouts feature-on-partitions:
  qT/kT/vT = W^T-tile.T @ xT-tile accumulated over 32 d-tiles (fp32r)
  RoPE applied in "rotate_half" form: weight rows are pre-permuted on
  the host (even features first, then odd) so the pair (2i, 2i+1)
  becomes (i, i+64) and the cross-partition shuffle is two 64-partition
  copies instead of a stride-2 partition gather.
  scores[tq,tk] per (b,h) via matmul over head_dim, +mask, softmax on
  the free axis, PE-transpose of probs, PV matmul, then the wo matmul
  over local features only (partial sums, bf16).
"""

import math
from contextlib import ExitStack

import ml_dtypes
import numpy as np

import concourse.bass as bass
import concourse.tile as tile
from concourse import bacc, mybir
from concourse.bass_utils import run_bass_kernel_spmd

DIM = 4096
N_HEADS = 32
HEAD_DIM = 128
N_KV_HEADS = 8
BSZ = 4
SEQLEN = 128
T = BSZ * SEQLEN  # 512 tokens
NCORES = 8
HQ = N_HEADS // NCORES  # 4 query heads per core
EQ = HQ * HEAD_DIM  # 512 local q features
ND = DIM // 128  # 32 contraction tiles
SCALE = 1.0 / math.sqrt(HEAD_DIM)

F32 = mybir.dt.float32
F32R = mybir.dt.float32r
BF16 = mybir.dt.bfloat16
AX = mybir.AxisListType
ACTF = mybir.ActivationFunctionType
PSUM = bass.MemorySpace.PSUM

_STATE: dict = {}
LAST_RESULT = None


def _install_ntff_hook():
    """Register the axon NTFF profile hook if the image lacks antenv.axon_hooks.

    Lets run_bass_kernel_spmd(trace=True) return exec_time_ns + perfetto
    under axon. Best-effort: any failure leaves tracing disabled but the
    kernel still runs.
    """
    import os
    import sys
    import types

    try:
        import antenv.axon_hooks  # noqa: F401

        return
    except ImportError:
        pass
    try:
        import antenv
        from trn_agent_boot.trn_boot import _ntff_profile_via_ctypes

        mod = types.ModuleType("antenv.axon_hooks")
        holder = {"hook": None}
        mod.set_axon_ntff_profile_hook = lambda h: holder.__setitem__("hook", h)
        mod.get_axon_ntff_profile_hook = lambda: holder["hook"]
        sys.modules["antenv.axon_hooks"] = mod
        antenv.axon_hooks = mod
        so = "/opt/axon/libaxon_pjrt.so"
        if os.path.exists(so):
            hook = _ntff_profile_via_ctypes(so)
            if hook is not None:
                mod.set_axon_ntff_profile_hook(hook)
    except Exception:
        pass


_install_ntff_hook()


def _build_nc(fast: bool):
    """Build the SPMD kernel graph.

    fast=False: x/wq/wk/wv in fp32r (full-precision q/k path — robust even
    when softmax logits are winner-take-all). fast=True: bf16 inputs (half
    the DMA bytes; fine when softmax is smooth).
    """
    XD = BF16 if fast else F32R
    QD = BF16 if fast else F32
    nc = bacc.Bacc(
        "TRN2",
        target_bir_lowering=False,
        debug=False,
        enable_asserts=False,
        num_devices=NCORES,
    )
    xT = nc.dram_tensor("xT", [DIM, T], XD, kind="ExternalInput").ap()
    # head-major: rows h*DIM + d, cols = the head's 128 features
    wqT = nc.dram_tensor("wqT", [HQ * DIM, HEAD_DIM], XD, kind="ExternalInput").ap()
    wkT = nc.dram_tensor("wkT", [DIM, HEAD_DIM], XD, kind="ExternalInput").ap()
    wvT = nc.dram_tensor("wvT", [DIM, HEAD_DIM], XD, kind="ExternalInput").ap()
    woT = nc.dram_tensor("woT", [EQ, DIM], BF16, kind="ExternalInput").ap()
    mask1 = nc.dram_tensor("mask1", [128, 128], F32, kind="ExternalInput").ap()
    cq = nc.dram_tensor("cq", [128, T], BF16, kind="ExternalInput").ap()
    sq = nc.dram_tensor("sq", [128, T], BF16, kind="ExternalInput").ap()
    ck = nc.dram_tensor("ck", [128, T], BF16, kind="ExternalInput").ap()
    sk = nc.dram_tensor("sk", [128, T], BF16, kind="ExternalInput").ap()
    ident = nc.dram_tensor("ident", [128, 128], BF16, kind="ExternalInput").ap()
    yT = nc.dram_tensor("yT", [DIM, T], BF16, kind="ExternalOutput").ap()

    with tile.TileContext(nc) as tc, ExitStack() as ctx:
        const = ctx.enter_context(tc.tile_pool(name="const", bufs=1))
        wp = ctx.enter_context(tc.tile_pool(name="wp", bufs=4))
        qtp = ctx.enter_context(tc.tile_pool(name="qtp", bufs=2))
        rt = ctx.enter_context(tc.tile_pool(name="rt", bufs=2))
        sm = ctx.enter_context(tc.tile_pool(name="sm", bufs=3))
        yp = ctx.enter_context(tc.tile_pool(name="yp", bufs=4))
        ps = ctx.enter_context(tc.tile_pool(name="ps", bufs=8, space=PSUM))

        # ---- PE warm-up: dummy matmuls to lift the HAM clock gate while
        # the first DMAs land ----
        warm_w = const.tile([128, 128], BF16, tag="warm_w")
        nc.gpsimd.memset(warm_w[:], 0.0)
        warm_x = const.tile([128, T], BF16, tag="warm_x")
        nc.gpsimd.memset(warm_x[:], 0.0)
        ps_warm = ps.tile([128, T], F32, tag="ps")
        for _ in range(10):
            nc.tensor.matmul(ps_warm[:], warm_w[:], warm_x[:], start=True, stop=True)

        # ---- input DMAs ----
        # Arrival order == consumption order: wk/wv first (K/V sweep), then
        # x groups striped across both HWDGE rings, then per-head wq, then
        # wo + tables. Separate tiles per transfer keep deps fine-grained.
        wk_sb = wp.tile([128, ND, HEAD_DIM], XD, tag="w", name="wk")
        nc.sync.dma_start(wk_sb[:], wkT.rearrange("(j p) e -> p j e", p=128))
        wv_sb = wp.tile([128, ND, HEAD_DIM], XD, tag="w", name="wv")
        nc.scalar.dma_start(wv_sb[:], wvT.rearrange("(j p) e -> p j e", p=128))

        XGROUPS = [2, 2, 4, 8, 8, 8]
        x_tiles = []
        j0 = 0
        for gi, gd in enumerate(XGROUPS):
            rows = slice(j0 * 128, (j0 + gd) * 128)
            xg = const.tile([128, gd, T], XD, tag=f"x{gi}", name=f"x{gi}")
            eng = nc.sync if gi % 2 == 0 else nc.scalar
            eng.dma_start(xg[:], xT[rows, :].rearrange("(j p) t -> p j t", p=128))
            for jj in range(gd):
                x_tiles.append((xg, jj))
            j0 += gd

        ident_sb = const.tile([128, 128], BF16, tag="ident")
        nc.scalar.dma_start(ident_sb[:], ident)
        ck_sb = const.tile([128, T], BF16, tag="ck")
        nc.scalar.dma_start(ck_sb[:], ck)
        sk_sb = const.tile([128, T], BF16, tag="sk")
        nc.scalar.dma_start(sk_sb[:], sk)

        wq_tiles = []
        for h in range(HQ):
            wqt = wp.tile([128, ND, HEAD_DIM], XD, tag="w", name=f"wq{h}")
            eng = nc.sync if h % 2 == 0 else nc.scalar
            eng.dma_start(
                wqt[:],
                wqT[h * DIM : (h + 1) * DIM, :].rearrange("(j p) e -> p j e", p=128),
            )
            wq_tiles.append(wqt)
            if h == 0:
                cq_sb = const.tile([128, T], BF16, tag="cq")
                nc.scalar.dma_start(cq_sb[:], cq)
                sq_sb = const.tile([128, T], BF16, tag="sq")
                nc.scalar.dma_start(sq_sb[:], sq)
                mask_sb = const.tile([128, 128], F32, tag="mask")
                nc.scalar.dma_start(mask_sb[:], mask1)
        wo_sb = const.tile([128, HQ * DIM], BF16, tag="wo")
        nc.sync.dma_start(
            wo_sb[:].rearrange("p (j c) -> p j c", j=HQ),
            woT.rearrange("(j p) c -> p j c", p=128),
        )

        kT_sb = const.tile([128, T], QD, tag="kT")
        vT_sb = const.tile([128, T], BF16, tag="vT")
        v_sb = const.tile([128, BSZ * HEAD_DIM], BF16, tag="v")
        oT_sb = const.tile([128, HQ * T], BF16, tag="oT")

        def rope(dst_ap, pssrc, ctab, stab):
            swp = rt.tile([128, T], F32, tag="swp")
            nc.scalar.copy(swp[0:64, :], pssrc[64:128, :])
            nc.scalar.copy(swp[64:128, :], pssrc[0:64, :])
            prod = rt.tile([128, T], F32, tag="prod")
            nc.vector.tensor_mul(prod[:], pssrc[:], ctab)
            nc.vector.tensor_mul(swp[:], swp[:], stab)
            nc.vector.tensor_add(dst_ap, prod[:], swp[:])

        # ---- K/V sweep (rides the incoming x stream) ----
        ps_k = ps.tile([128, T], F32, tag="ps")
        ps_v = ps.tile([128, T], F32, tag="ps")
        for j in range(ND):
            st, sp = (j == 0), (j == ND - 1)
            xg, jj = x_tiles[j]
            nc.tensor.matmul(ps_k[:], wk_sb[:, j, :], xg[:, jj, :], start=st, stop=sp)
            nc.tensor.matmul(ps_v[:], wv_sb[:, j, :], xg[:, jj, :], start=st, stop=sp)
        rope(kT_sb[:], ps_k[:], ck_sb[:], sk_sb[:])
        nc.scalar.copy(vT_sb[:], ps_v[:])
        for b in range(BSZ):
            bs = slice(b * 128, (b + 1) * 128)
            ps_t = ps.tile([128, T], BF16, tag="ps")
            nc.tensor.transpose(ps_t[:, 0:128], vT_sb[:, bs], ident_sb[:])
            nc.vector.tensor_copy(v_sb[:, bs], ps_t[:, 0:128])

        # ---- per-head Q sweep; attention split into a scores part and a
        # probs/PV part, staggered across the following sweeps so the PE
        # never waits on softmax latency ----
        def q_sweep(h):
            ps_qh = ps.tile([128, T], F32, tag="ps", name=f"ps_q{h}")
            wqt = wq_tiles[h]
            for j in range(ND):
                st, sp = (j == 0), (j == ND - 1)
                xg, jj = x_tiles[j]
                nc.tensor.matmul(
                    ps_qh[:], wqt[:, j, :], xg[:, jj, :], start=st, stop=sp
                )
            qt = qtp.tile([128, T], QD, tag="qT", name=f"qT{h}")
            rope(qt[:], ps_qh[:], cq_sb[:], sq_sb[:])
            return qt

        def att_scores(h, qt):
            ps_s = ps.tile([128, T], F32, tag="ps", name=f"ps_s{h}")
            for b in range(BSZ):
                bs = slice(b * 128, (b + 1) * 128)
                nc.tensor.matmul(
                    ps_s[:, bs], qt[:, bs], kT_sb[:, bs], start=True, stop=True
                )
            s_sb = sm.tile([128, T], F32, tag="s", name=f"s{h}")
            nmx = sm.tile([128, BSZ], F32, tag="nmx", name=f"nmx{h}")
            den = sm.tile([128, BSZ], F32, tag="den", name=f"den{h}")
            rden = sm.tile([128, BSZ], F32, tag="rden", name=f"rden{h}")
            p_sb = sm.tile([128, T], BF16, tag="p", name=f"p{h}")
            for b in range(BSZ):
                bs = slice(b * 128, (b + 1) * 128)
                nc.vector.tensor_add(s_sb[:, bs], ps_s[:, bs], mask_sb[:])
                nc.vector.reduce_max(
                    nmx[:, b : b + 1], s_sb[:, bs], axis=AX.X, negate=True
                )
                nc.scalar.activation(
                    p_sb[:, bs],
                    s_sb[:, bs],
                    ACTF.Exp,
                    bias=nmx[:, b : b + 1],
                    accum_out=den[:, b : b + 1],
                )
            nc.vector.reciprocal(rden[:], den[:])
            for b in range(BSZ):
                bs = slice(b * 128, (b + 1) * 128)
                nc.vector.tensor_scalar_mul(p_sb[:, bs], p_sb[:, bs], rden[:, b : b + 1])
            return p_sb

        def att_pv(h, p_sb):
            for b in range(BSZ):
                bs = slice(b * 128, (b + 1) * 128)
                ps_pt = ps.tile([128, T], BF16, tag="ps", name=f"ps_pt{h}_{b}")
                nc.tensor.transpose(ps_pt[:, 0:128], p_sb[:, bs], ident_sb[:])
                pt_sb = sm.tile([128, 128], BF16, tag="pt", name=f"pt{h}_{b}")
                nc.scalar.copy(pt_sb[:], ps_pt[:, 0:128])
                ps_o = ps.tile([128, T], F32, tag="ps", name=f"ps_o{h}_{b}")
                nc.tensor.matmul(
                    ps_o[:, 0:128], v_sb[:, bs], pt_sb[:], start=True, stop=True
                )
                nc.vector.tensor_copy(
                    oT_sb[:, h * T + b * 128 : h * T + (b + 1) * 128], ps_o[:, 0:128]
                )

        qts = {}
        probs = {}
        for h in range(HQ):
            qts[h] = q_sweep(h)
            if h >= 1:
                probs[h - 1] = att_scores(h - 1, qts[h - 1])
            if h >= 2:
                att_pv(h - 2, probs[h - 2])
        probs[HQ - 1] = att_scores(HQ - 1, qts[HQ - 1])
        att_pv(HQ - 2, probs[HQ - 2])
        att_pv(HQ - 1, probs[HQ - 1])

        # ---- output projection over local features (partial sums) ----
        for dt in range(ND):
            ps_y = ps.tile([128, T], F32, tag="ps", name=f"ps_y{dt}")
            for j in range(HQ):
                nc.tensor.matmul(
                    ps_y[:],
                    wo_sb[:, j * DIM + dt * 128 : j * DIM + (dt + 1) * 128],
                    oT_sb[:, j * T : (j + 1) * T],
                    start=(j == 0),
                    stop=(j == HQ - 1),
                )
            y_sb = yp.tile([128, T], BF16, tag="y", name=f"y{dt}")
            if dt % 2 == 0:
                nc.vector.tensor_copy(y_sb[:], ps_y[:])
                nc.sync.dma_start(yT[dt * 128 : (dt + 1) * 128, :], y_sb[:])
            else:
                nc.scalar.copy(y_sb[:], ps_y[:])
                nc.scalar.dma_start(yT[dt * 128 : (dt + 1) * 128, :], y_sb[:])

    nc.compile()
    return nc


def get_nc(fast: bool):
    key = "nc_fast" if fast else "nc_robust"
    if key not in _STATE:
        _STATE[key] = _build_nc(fast)
    return _STATE[key]


def _prep_in_maps(x, wq, wk, wv, wo, freqs_cos, freqs_sin, mask, fast):
    f32 = np.float32
    bf16 = ml_dtypes.bfloat16
    xd = bf16 if fast else f32
    x = np.asarray(x, f32)
    wq = np.asarray(wq, f32)
    wk = np.asarray(wk, f32)
    wv = np.asarray(wv, f32)
    wo = np.asarray(wo, f32)
    fc = np.asarray(freqs_cos, f32)
    fs = np.asarray(freqs_sin, f32)
    mask = np.asarray(mask, f32)

    # even features first, then odd: (2i, 2i+1) pairs -> (i, i+64)
    perm = np.concatenate([np.arange(0, HEAD_DIM, 2), np.arange(1, HEAD_DIM, 2)])
    wqp = wq.reshape(N_HEADS, HEAD_DIM, DIM)[:, perm, :].reshape(DIM, DIM)
    wkp = wk.reshape(N_KV_HEADS, HEAD_DIM, DIM)[:, perm, :].reshape(
        N_KV_HEADS * HEAD_DIM, DIM
    )

    xT = np.ascontiguousarray(x.reshape(T, DIM).T).astype(xd)
    C0 = np.vstack([fc.T, fc.T])  # [128, 128]: row p -> cos[t, p % 64]
    S0 = np.vstack([-fs.T, fs.T])
    cq = np.ascontiguousarray(np.tile(C0 * SCALE, (1, BSZ))).astype(bf16)
    sq = np.ascontiguousarray(np.tile(S0 * SCALE, (1, BSZ))).astype(bf16)
    ck = np.ascontiguousarray(np.tile(C0, (1, BSZ))).astype(bf16)
    sk = np.ascontiguousarray(np.tile(S0, (1, BSZ))).astype(bf16)
    mask1 = np.ascontiguousarray(mask[0, 0])
    ident = np.eye(128, dtype=bf16)

    in_maps = []
    for c in range(NCORES):
        qrows = slice(c * EQ, (c + 1) * EQ)
        krows = slice(c * HEAD_DIM, (c + 1) * HEAD_DIM)
        in_maps.append(
            {
                "xT": xT,
                "wqT": np.ascontiguousarray(
                    wqp[qrows, :]
                    .reshape(HQ, HEAD_DIM, DIM)
                    .transpose(0, 2, 1)
                    .reshape(HQ * DIM, HEAD_DIM)
                ).astype(xd),
                "wkT": np.ascontiguousarray(wkp[krows, :].T).astype(xd),
                "wvT": np.ascontiguousarray(wv[krows, :].T).astype(xd),
                "woT": np.ascontiguousarray(wo[:, qrows].T).astype(bf16),
                "mask1": mask1,
                "cq": cq,
                "sq": sq,
                "ck": ck,
                "sk": sk,
                "ident": ident,
            }
        )
    return in_maps


def _pick_fast(x, wq):
    """bf16 q/k only when softmax logits are smooth (score sigma small).

    score_sigma ~= std(x) * std(wq) * sqrt(DIM * HEAD_DIM) * SCALE. In the
    winner-take-all regime (sigma >> 1) bf16 rounding flips argmaxes, so use
    the fp32r path there.
    """
    sx = float(np.asarray(x, np.float32).std())
    sw = float(np.asarray(wq, np.float32).std())
    sigma = sx * sw * math.sqrt(DIM * HEAD_DIM) * SCALE
    return sigma < 8.0


def kernel(
    x,
    wq,
    wk,
    wv,
    wo,
    cache_k,
    cache_v,
    freqs_cos,
    freqs_sin,
    mask,
    start_pos,
    *,
    trace=False,
    trace_kwargs=None,
):
    global LAST_RESULT
    sp = int(np.asarray(start_pos))
    assert sp == 0, f"kernel specialized for start_pos=0, got {sp}"

    fast = _pick_fast(x, wq)
    in_maps = _prep_in_maps(x, wq, wk, wv, wo, freqs_cos, freqs_sin, mask, fast)
    nc = get_nc(fast)
    res = run_bass_kernel_spmd(
        nc,
        in_maps,
        core_ids=list(range(NCORES)),
        trace=trace,
        **(trace_kwargs or {}),
    )
    LAST_RESULT = res
    acc = np.zeros((DIM, T), np.float32)
    for c in range(NCORES):
        acc += res.results[c]["yT"].astype(np.float32)
    return np.ascontiguousarray(acc.T).reshape(BSZ, SEQLEN, DIM)
